# revision 1
# baseline (speedup 1.0000x reference)
"""CTConv2d Trainium2 kernel.

Computes y = conv2d(x, w) where w (O,I,3,3) is synthesized on host from
core/periphery/threshold/scale (tiny tensors), and the conv runs on 8
NeuronCores, data-parallel over batch (32 images -> 4 per core).

Device kernel (per core): hybrid PE/DVE decomposition of the 3x3 conv.
All 9 taps share the same channel-mix matrix up to a scalar (the
synthesized weight is w[o,i,kh,kw] = gate-scaled core x periphery), so
the vertical tap pair (-1,0)/(+1,0) and horizontal pair (0,-1)/(0,+1)
are each collapsed on the Vector engine into one fused op
    z = (x_shifted_a * ratio) + x_shifted_b
with the pair's base coefficient folded into that pair's matmul matrix.
Per 4-row output block the PE then runs 7 accumulating fp16 matmuls
(center + 4 corner taps + z_vert + z_horiz, K=128 channels, N=448)
into fp32 PSUM instead of 9 -> ~22% less PE time, the kernel's
bottleneck. Ratios are runtime data (per-partition scalars), so the
compiled NEFF stays valid for any input values.

Layout: host bakes x into a zero-padded (114 x 116) fp16 image per
channel (interior at row 1, col 2; stride 116 keeps dw=0 reads 4-byte
aligned so the vertical fused op hits the DVE 2x mode). Image loads are
contiguous row-chunks on the SP HWDGE ring; output DMAs go on the ACT
HWDGE ring. Accumulation is fp32; inputs rounded to fp16 (~3e-4
relative absmax vs the fp32 reference).
"""

import os
import sys

# The grading/bench environment may pin JAX_PLATFORMS=cpu for the jax
# reference; this kernel needs the axon/neuron PJRT backend.
if os.environ.get("JAX_PLATFORMS") == "cpu":
    del os.environ["JAX_PLATFORMS"]

for _p in ("/opt/trn_rl_repo",):
    if os.path.isdir(_p) and _p not in sys.path:
        sys.path.append(_p)

import numpy as np

import concourse.bass as bass
import concourse.mybir as mybir
from concourse import bacc
from concourse.bass_utils import run_bass_kernel_spmd
from concourse.tile import TileContext

O = 128
I = 128
B = 32
H = 112
W = 112
NCORES = 8
BPC = B // NCORES  # images per core
HP = H + 2  # padded rows (interior at row 1)
WP = W + 4  # padded cols, stride 116 (interior at col 2)
C0 = 2  # interior column offset
RB = 4  # output rows per PSUM group (N = RB*W = 448 <= 512)
NBLK = H // RB  # 28
GRP = 4  # PSUM groups / z-piece rows (16) per output DMA
ZROWS = GRP * RB  # 16 output rows per z piece
NP = NBLK // GRP  # 7 pieces per image
# image-load chunks in padded-row units (contiguous); first chunk is
# small so the first piece's fused z ops (which need padded rows
# [0, rows+2)) are ready almost immediately.
CHUNKS = [(0, 6), (6, 18), (18, 50), (50, 82), (82, HP)]
# output blocks per z piece; small early pieces so the PE's first
# z-matmuls aren't gated on big DVE ops while the pipeline fills.
PIECE_BLOCKS = [1, 1, 2, 4, 4, 4, 4, 4, 4]
# output-DMA group sizes (blocks per out tile); tapered at the end so
# the final copy+DMA chain after the last matmul is short.
OUT_GROUPS = [4, 4, 4, 4, 4, 4, 2, 1, 1]
F32 = mybir.dt.float32
F16 = mybir.dt.float16

# PE taps: 4 corners + center; then z_vert, z_horiz
PE_TAPS = [(-1, -1), (-1, 1), (0, 0), (1, -1), (1, 1)]
NW = len(PE_TAPS) + 2  # 7 weight matrices

EPS_FOLD = 1e-7


def synth_weights(core, periphery, threshold, scale):
    """Host-side weight synthesis for the hybrid decomposition.

    Returns (wmat, ratios):
      wmat (I, 7*O) fp16, lhsT layout wmat[i, t*O+o]:
        t=0..4: corner/center taps from PE_TAPS,
        t=5: p_down * CG (vertical pair base, fold of x[h-1] ratio),
        t=6: p_right * CG (horizontal pair base).
      ratios (rv, rh) float: z_v = rv*x[h-1] + x[h+1],
                             z_h = rh*x[:,w-1] + x[:,w+1].
    """
    c = np.asarray(core, np.float64)[:, :, 0, 0]  # (O, I)
    thr = np.asarray(threshold, np.float64)  # (O,)
    s = float(np.asarray(scale, np.float64)[0])
    p = np.asarray(periphery, np.float64)  # (8,)
    gate = 1.0 / (1.0 + np.exp(-s * (np.abs(c) - thr[:, None])))  # (O, I)
    p_full = np.concatenate([p[:4], [1.0], p[4:]])  # (9,) taps row-major
    cg = c * gate

    def ptap(dh, dw):
        return p_full[(dh + 1) * 3 + (dw + 1)]

    w = np.empty((NW, O, I), np.float64)
    for t, (dh, dw) in enumerate(PE_TAPS):
        w[t] = c if (dh, dw) == (0, 0) else cg * ptap(dh, dw)

    def fold(p_a, p_b):
        # z = ratio*x_a + x_b, matmul matrix = p_b_clamped * CG;
        # p_b clamped away from zero so ratio*p_b == p_a exactly.
        pb = p_b if abs(p_b) >= EPS_FOLD else (EPS_FOLD if p_b >= 0 else -EPS_FOLD)
        return p_a / pb, pb

    rv, pv = fold(ptap(-1, 0), ptap(1, 0))  # a = x[h-1], b = x[h+1]
    rh, ph = fold(ptap(0, -1), ptap(0, 1))  # a = x[:,w-1], b = x[:,w+1]
    w[5] = cg * pv
    w[6] = cg * ph

    wmat = np.ascontiguousarray(w.transpose(2, 0, 1)).reshape(I, NW * O)
    return np.ascontiguousarray(wmat.astype(np.float16)), (rv, rh)


def build_nc():
    nc = bacc.Bacc(None)
    x_d = nc.dram_tensor("x", [BPC, I, HP * WP], F16, kind="ExternalInput")
    w_d = nc.dram_tensor("w", [I, NW * O], F16, kind="ExternalInput")
    r_d = nc.dram_tensor("r", [128, 2], F32, kind="ExternalInput")
    y_d = nc.dram_tensor("y", [BPC, O, H, W], F32, kind="ExternalOutput")

    mult = mybir.AluOpType.mult
    add = mybir.AluOpType.add

    with TileContext(nc) as tc, tc.tile_pool(name="persist", bufs=1) as persist:
        wt = persist.tile([I, NW * O], F16, name="wt", tag="wt")
        nc.sync.dma_start(out=wt[:], in_=w_d[:])
        rt = persist.tile([128, 2], F32, name="rt", tag="rt")

        imgs = []
        for ib in range(2):
            t = persist.tile([128, HP * WP], F16, name=f"img{ib}", tag=f"img{ib}")
            imgs.append(t)

        # HAM warmup: the PE clock gate sits at 1.2 GHz until ~3.4us of
        # sustained matmul activity. A dependency-free burst right after
        # engine boot flips it to 2.4 GHz before the first real matmul
        # (which waits ~5us on the weight/chunk DMA receipt chain anyway).
        warm = persist.tile([128, 640], F16, name="warm", tag="warm")
        nc.vector.memset(warm[:], 0.0)

        def load_image(b, first=False):
            img = imgs[b % 2]
            for ci, (r0, r1) in enumerate(CHUNKS):
                # image 0's first chunks go on the ACT HWDGE ring so their
                # DMA receipt overlaps the weight load's on the SP ring.
                eng = nc.scalar if first and ci < 2 else nc.sync
                eng.dma_start(
                    out=img[:, r0 * WP : r1 * WP],
                    in_=x_d[b][:, r0 * WP : r1 * WP],
                )

        with (
            tc.tile_pool(name="psum", bufs=8, space="PSUM") as psum_pool,
            tc.tile_pool(name="outp", bufs=3) as out_pool,
            tc.tile_pool(name="zp", bufs=3) as z_pool,
        ):
            # block index -> piece start block (for z tile offsets)
            piece_start = {}
            blk0 = 0
            for nb in PIECE_BLOCKS:
                for j in range(nb):
                    piece_start[blk0 + j] = (blk0, nb)
                blk0 += nb
            assert blk0 == NBLK
            # block index -> (group start block, group size)
            group_of = {}
            blk0 = 0
            for ng in OUT_GROUPS:
                for j in range(ng):
                    group_of[blk0 + j] = (blk0, ng)
                blk0 += ng
            assert blk0 == NBLK

            for k in range(10):
                pw = psum_pool.tile([128, 512], F32, name="pw", tag="ps")
                nc.tensor.matmul(
                    out=pw[:],
                    lhsT=warm[:, 0:128],
                    rhs=warm[:, 128:640],
                    start=True,
                    stop=True,
                )
            load_image(0, first=True)
            # ratios are only needed by the DVE z ops; keep this tiny DMA
            # off the critical first-chunk path.
            nc.sync.dma_start(out=rt[:], in_=r_d[:])
            for b in range(BPC):
                if b + 1 < BPC:
                    load_image(b + 1)
                img3 = imgs[b % 2].rearrange("p (h w) -> p h w", w=WP)
                yflat = y_d[b].rearrange("o h w -> o (h w)")
                zv = zh = None
                ot = None
                for blk in range(NBLK):
                    p0, pnb = piece_start[blk]
                    if blk == p0:
                        # fused pair ops on DVE for this piece's rows
                        hz = p0 * RB  # first output row
                        nr = pnb * RB  # rows in piece
                        zv = z_pool.tile([128, nr * W], F16, name="zv", tag="zv")
                        zh = z_pool.tile([128, nr * W], F16, name="zh", tag="zh")
                        zv3 = zv.rearrange("p (h w) -> p h w", w=W)
                        zh3 = zh.rearrange("p (h w) -> p h w", w=W)
                        # padded row of output row h is h+1
                        nc.vector.scalar_tensor_tensor(
                            out=zv3[:, :, :],
                            in0=img3[:, hz : hz + nr, C0 : C0 + W],
                            scalar=rt[:, 0:1],
                            in1=img3[:, hz + 2 : hz + 2 + nr, C0 : C0 + W],
                            op0=mult,
                            op1=add,
                        )
                        nc.vector.scalar_tensor_tensor(
                            out=zh3[:, :, :],
                            in0=img3[:, hz + 1 : hz + 1 + nr, C0 - 1 : C0 - 1 + W],
                            scalar=rt[:, 1:2],
                            in1=img3[:, hz + 1 : hz + 1 + nr, C0 + 1 : C0 + 1 + W],
                            op0=mult,
                            op1=add,
                        )
                    g0, gsz = group_of[blk]
                    if blk == g0:
                        ot = out_pool.tile(
                            [128, gsz * RB * W], F32, name="ot", tag="ot"
                        )
                    h0 = blk * RB
                    ps = psum_pool.tile([128, RB * W], F32, name="ps")
                    for ti, (dh, dw) in enumerate(PE_TAPS):
                        rhs = img3[
                            :,
                            h0 + 1 + dh : h0 + 1 + dh + RB,
                            C0 + dw : C0 + dw + W,
                        ]
                        nc.tensor.matmul(
                            out=ps[:],
                            lhsT=wt[:, ti * O : (ti + 1) * O],
                            rhs=rhs,
                            start=(ti == 0),
                            stop=False,
                        )
                    zoff = (blk - p0) * RB * W
                    nc.tensor.matmul(
                        out=ps[:],
                        lhsT=wt[:, 5 * O : 6 * O],
                        rhs=zv[:, zoff : zoff + RB * W],
                        start=False,
                        stop=False,
                    )
                    nc.tensor.matmul(
                        out=ps[:],
                        lhsT=wt[:, 6 * O : 7 * O],
                        rhs=zh[:, zoff : zoff + RB * W],
                        start=False,
                        stop=True,
                    )
                    joff = (blk - g0) * RB * W
                    nc.scalar.copy(out=ot[:, joff : joff + RB * W], in_=ps[:])
                    if blk == g0 + gsz - 1:
                        n = RB * W
                        nc.scalar.dma_start(
                            out=yflat[:, g0 * n : (g0 + gsz) * n], in_=ot[:]
                        )
    nc.finalize()
    return nc


_NC_CACHE = {}


def _get_nc():
    if "nc" not in _NC_CACHE:
        _NC_CACHE["nc"] = build_nc()
    return _NC_CACHE["nc"]


def _pad_images(x):
    """(B, I, H, W) fp32 -> (B, I, HP*WP) fp16, zero halo baked in."""
    xp = np.zeros((B, I, HP, WP), np.float16)
    xp[:, :, 1 : 1 + H, C0 : C0 + W] = x.astype(np.float16)
    return np.ascontiguousarray(xp.reshape(B, I, HP * WP))


def run(inputs, trace=False, **kw):
    """Run on hardware; returns (y, BassKernelResults)."""
    x = np.asarray(inputs["x"], np.float32)
    assert x.shape == (B, I, H, W), x.shape
    wmat, (rv, rh) = synth_weights(
        inputs["core"], inputs["periphery"], inputs["threshold"], inputs["scale"]
    )
    xp = _pad_images(x)
    ratios = np.empty((128, 2), np.float32)
    ratios[:, 0] = rv
    ratios[:, 1] = rh
    nc = _get_nc()
    in_maps = [
        {"x": xp[c * BPC : (c + 1) * BPC], "w": wmat, "r": ratios}
        for c in range(NCORES)
    ]
    res = run_bass_kernel_spmd(nc, in_maps, list(range(NCORES)), trace=trace, **kw)
    y = np.concatenate([res.results[c]["y"] for c in range(NCORES)], axis=0)
    return y, res


def kernel(**inputs) -> np.ndarray:
    y, _ = run(inputs)
    return y



# revision 2
# speedup vs baseline: 1.2403x; 1.2403x over previous
"""CTConv2d Trainium2 kernel.

Computes y = conv2d(x, w) where w (O,I,3,3) is synthesized on host from
core/periphery/threshold/scale (tiny tensors), and the conv runs on 8
NeuronCores, data-parallel over batch (32 images -> 4 per core).

Device kernel (per core): hybrid PE decomposition of the 3x3 conv using
fp8 DoubleRow matmuls. The synthesized weight is
    w[o,i,dh,dw] = center: c[o,i]; else c*gate (cg) * p[dh,dw],
so per 4-row output block the PE runs:
  - 1 fp16 matmul for the center tap (accuracy-critical, ~93% of energy)
  - 3 fp8e4 DoubleRow matmuls, each covering a vertical tap PAIR
    (dh=-1,dw)+(dh=+1,dw) for dw in {-1,0,+1}: DoubleRow packs 2 fp8
    weights per PE cell (virtual K=256) so each pair costs one pass.
    The moving operand is a hand-built overlapping 4D access pattern
    [pair@2*WP8, h@WP8, w@1] over the fp8 image.
  - 1 fp16 matmul for the horizontal pair, DVE-fused as
    zh = (p(0,-1)/p(0,+1))*x[w-1] + x[w+1]  (p(0,+1) folded into the
    matmul weight). The fp16 image interior sits at an ODD column
    offset so both zh operands are 4-byte aligned -> DVE 2x mode.
All weights are pre-scaled by S=1024 so the tiny fp8 weights (~4e-3)
land in e4m3's normal range; the PSUM->SBUF copy on the scalar engine
compensates with its free affine scale (1/S) and emits fp16, halving
the output DMA.

Inputs DMA'd per image: fp16 padded image (stride 118, interior col 3)
for center/zh, fp8e4 padded image (stride 120, interior col 4) for the
DoubleRow taps. Accumulation is fp32 in PSUM; absmax rel err ~1e-3.
"""

import os
import sys

# The grading/bench environment may pin JAX_PLATFORMS=cpu for the jax
# reference; this kernel needs the axon/neuron PJRT backend.
if os.environ.get("JAX_PLATFORMS") == "cpu":
    del os.environ["JAX_PLATFORMS"]

for _p in ("/opt/trn_rl_repo",):
    if os.path.isdir(_p) and _p not in sys.path:
        sys.path.append(_p)

import numpy as np
import ml_dtypes
import bass_rust

import concourse.bass as bass
import concourse.mybir as mybir
from concourse import bacc
from concourse.bass_utils import run_bass_kernel_spmd
from concourse.tile import TileContext

O = 128
I = 128
B = 32
H = 112
W = 112
NCORES = 8
BPC = B // NCORES  # images per core
HP = H + 2  # padded rows (interior at row 1)
WP16 = 118  # fp16 image row stride; interior at col C16 (odd -> zh aligned)
C16 = 3
WP8 = 120  # fp8 image row stride (2*WP8 % 16 == 0 for DR pair step)
C8 = 4
RB = 4  # output rows per PSUM group (N = RB*W = 448 <= 512)
NBLK = H // RB  # 28
GRP = 4  # PSUM groups / zh-piece rows (16) per output DMA
NP = NBLK // GRP  # 7 pieces per image
SCL = 1024.0  # weight pre-scale so fp8 weights sit in e4m3 normal range
# image-load chunks in padded-row units (contiguous); first chunk is
# small so the first piece's ops are ready almost immediately.
CHUNKS = [(0, 6), (6, 18), (18, 50), (50, 82), (82, HP)]
# output blocks per zh piece; small early pieces so the PE's first
# zh-matmuls aren't gated on big DVE ops while the pipeline fills.
PIECE_BLOCKS = [1, 1, 2, 4, 4, 4, 4, 4, 4]
# output-DMA group sizes (blocks per out tile); tapered at the end so
# the final copy+DMA chain after the last matmul is short.
OUT_GROUPS = [4, 4, 4, 4, 4, 4, 2, 1, 1]
F32 = mybir.dt.float32
F16 = mybir.dt.float16
F8 = mybir.dt.float8e4

DWS = (-1, 0, 1)  # the three DoubleRow vertical-pair columns

EPS_FOLD = 1e-7


def synth_weights(core, periphery, threshold, scale):
    """Host-side weight synthesis.

    Returns (w16, w8, rh):
      w16 (I, 2*O) fp16, lhsT layout: block 0 = center c*S,
        block 1 = cg*p(0,+1)*S (horizontal-pair fold).
      w8 (I, 3*2*O) fp8e4, lhsT layout per dw block t:
        [t*2*O + pair*O + o], pair 0 = dh=-1, pair 1 = dh=+1, cg*p*S.
      rh: zh ratio p(0,-1)/p(0,+1) (runtime data for the DVE op).
    """
    c = np.asarray(core, np.float64)[:, :, 0, 0]  # (O, I)
    thr = np.asarray(threshold, np.float64)  # (O,)
    s = float(np.asarray(scale, np.float64)[0])
    p = np.asarray(periphery, np.float64)  # (8,)
    gate = 1.0 / (1.0 + np.exp(-s * (np.abs(c) - thr[:, None])))  # (O, I)
    p_full = np.concatenate([p[:4], [1.0], p[4:]])  # (9,) taps row-major
    cg = c * gate

    def ptap(dh, dw):
        return p_full[(dh + 1) * 3 + (dw + 1)]

    ph = ptap(0, 1)
    if abs(ph) < EPS_FOLD:
        ph = EPS_FOLD if ph >= 0 else -EPS_FOLD
    rh = ptap(0, -1) / ph

    w16 = np.empty((2, O, I), np.float64)
    w16[0] = c * SCL
    w16[1] = cg * ph * SCL
    w16 = np.ascontiguousarray(w16.transpose(2, 0, 1).reshape(I, 2 * O))

    w8 = np.empty((3, 2, O, I), np.float64)
    for t, dw in enumerate(DWS):
        w8[t, 0] = cg * ptap(-1, dw) * SCL
        w8[t, 1] = cg * ptap(+1, dw) * SCL
    w8 = w8.transpose(3, 0, 1, 2).reshape(I, 3 * 2 * O)
    w8 = np.clip(w8, -240.0, 240.0)
    return (
        np.ascontiguousarray(w16.astype(np.float16)),
        np.ascontiguousarray(w8.astype(ml_dtypes.float8_e4m3)),
        rh,
    )


def _dr_rhs(img8v, h0, dw):
    """Overlapping 4D AP [pair@2*WP8, h@WP8, w@1] at padded row h0,
    col C8+dw: the moving operand for one DoubleRow vertical-pair MM."""
    c0 = C8 + dw
    sl = img8v[:, h0 : h0 + RB + 2, c0 : c0 + W]  # covers all bytes read
    rhs = sl.copy()
    part = list(rhs.ap[0])
    rhs.ap = bass_rust.VecI64Pair(
        [part, [2 * WP8, 2], [WP8, RB], [1, W]]
    )
    return rhs


def build_nc():
    nc = bacc.Bacc(None)
    x16_d = nc.dram_tensor("x16", [BPC, I, HP * WP16], F16, kind="ExternalInput")
    x8_d = nc.dram_tensor("x8", [BPC, I, HP * WP8], F8, kind="ExternalInput")
    w16_d = nc.dram_tensor("w16", [I, 2 * O], F16, kind="ExternalInput")
    w8_d = nc.dram_tensor("w8", [I, 3 * 2 * O], F8, kind="ExternalInput")
    r_d = nc.dram_tensor("r", [128, 1], F32, kind="ExternalInput")
    y_d = nc.dram_tensor("y", [BPC, O, H, W], F16, kind="ExternalOutput")

    mult = mybir.AluOpType.mult
    add = mybir.AluOpType.add

    with TileContext(nc) as tc, tc.tile_pool(name="persist", bufs=1) as persist:
        w16t = persist.tile([I, 2 * O], F16, name="w16t", tag="w16t")
        w8t = persist.tile([I, 3 * 2 * O], F8, name="w8t", tag="w8t")
        nc.sync.dma_start(out=w16t[:], in_=w16_d[:])
        nc.sync.dma_start(out=w8t[:], in_=w8_d[:])
        rt = persist.tile([128, 1], F32, name="rt", tag="rt")

        imgs16 = []
        imgs8 = []
        for ib in range(2):
            imgs16.append(
                persist.tile([128, HP * WP16], F16, name=f"i16_{ib}", tag=f"i16_{ib}")
            )
            imgs8.append(
                persist.tile([128, HP * WP8], F8, name=f"i8_{ib}", tag=f"i8_{ib}")
            )

        # HAM warmup: the PE clock gate sits at 1.2 GHz until ~3.4us of
        # sustained matmul activity. A dependency-free burst right after
        # engine boot flips it to 2.4 GHz before the first real matmul
        # (which waits ~5us on the weight/chunk DMA receipt chain anyway).
        warm = persist.tile([128, 640], F16, name="warm", tag="warm")
        nc.vector.memset(warm[:], 0.0)

        def load_image(b, first=False):
            i16 = imgs16[b % 2]
            i8 = imgs8[b % 2]
            for ci, (r0, r1) in enumerate(CHUNKS):
                # fp16 chunks on the SP HWDGE ring, fp8 on the ACT ring
                # (which also carries output DMAs, issued later).
                nc.sync.dma_start(
                    out=i16[:, r0 * WP16 : r1 * WP16],
                    in_=x16_d[b][:, r0 * WP16 : r1 * WP16],
                )
                nc.scalar.dma_start(
                    out=i8[:, r0 * WP8 : r1 * WP8],
                    in_=x8_d[b][:, r0 * WP8 : r1 * WP8],
                )

        with (
            tc.tile_pool(name="psum", bufs=8, space="PSUM") as psum_pool,
            tc.tile_pool(name="outp", bufs=3) as out_pool,
            tc.tile_pool(name="zp", bufs=3) as z_pool,
        ):
            # block index -> piece start block (for zh tile offsets)
            piece_start = {}
            blk0 = 0
            for nb in PIECE_BLOCKS:
                for j in range(nb):
                    piece_start[blk0 + j] = (blk0, nb)
                blk0 += nb
            assert blk0 == NBLK
            # block index -> (group start block, group size)
            group_of = {}
            blk0 = 0
            for ng in OUT_GROUPS:
                for j in range(ng):
                    group_of[blk0 + j] = (blk0, ng)
                blk0 += ng
            assert blk0 == NBLK

            for k in range(10):
                pw = psum_pool.tile([128, 512], F32, name="pw", tag="ps")
                nc.tensor.matmul(
                    out=pw[:],
                    lhsT=warm[:, 0:128],
                    rhs=warm[:, 128:640],
                    start=True,
                    stop=True,
                )
            load_image(0, first=True)
            # the ratio is only needed by the DVE zh ops; keep this tiny
            # DMA off the critical first-chunk path.
            nc.sync.dma_start(out=rt[:], in_=r_d[:])
            for b in range(BPC):
                if b + 1 < BPC:
                    load_image(b + 1)
                img16v = imgs16[b % 2].rearrange("p (h w) -> p h w", w=WP16)
                img8v = imgs8[b % 2].rearrange("p (h w) -> p h w", w=WP8)
                yflat = y_d[b].rearrange("o h w -> o (h w)")
                zh = None
                ot = None
                for blk in range(NBLK):
                    p0, pnb = piece_start[blk]
                    if blk == p0:
                        # horizontal-pair fuse on DVE for this piece's rows
                        hz = p0 * RB  # first output row
                        nr = pnb * RB  # rows in piece
                        zh = z_pool.tile([128, nr * W], F16, name="zh", tag="zh")
                        zh3 = zh.rearrange("p (h w) -> p h w", w=W)
                        # padded row of output row h is h+1; both operand
                        # column starts (C16-1, C16+1) are even -> 2x mode
                        nc.vector.scalar_tensor_tensor(
                            out=zh3[:, :, :],
                            in0=img16v[:, hz + 1 : hz + 1 + nr, C16 - 1 : C16 - 1 + W],
                            scalar=rt[:, 0:1],
                            in1=img16v[:, hz + 1 : hz + 1 + nr, C16 + 1 : C16 + 1 + W],
                            op0=mult,
                            op1=add,
                        )
                    g0, gsz = group_of[blk]
                    if blk == g0:
                        ot = out_pool.tile(
                            [128, gsz * RB * W], F16, name="ot", tag="ot"
                        )
                    h0 = blk * RB
                    ps = psum_pool.tile([128, RB * W], F32, name="ps")
                    nc.tensor.matmul(
                        out=ps[:],
                        lhsT=w16t[:, 0:O],
                        rhs=img16v[:, h0 + 1 : h0 + 1 + RB, C16 : C16 + W],
                        start=True,
                        stop=False,
                    )
                    for t, dw in enumerate(DWS):
                        nc.tensor.matmul(
                            out=ps[:],
                            lhsT=w8t[:, t * 2 * O : (t + 1) * 2 * O].rearrange(
                                "p (pair o) -> p pair o", pair=2
                            ),
                            rhs=_dr_rhs(img8v, h0, dw),
                            start=False,
                            stop=False,
                            perf_mode=mybir.MatmulPerfMode.DoubleRow,
                        )
                    zoff = (blk - p0) * RB * W
                    nc.tensor.matmul(
                        out=ps[:],
                        lhsT=w16t[:, O : 2 * O],
                        rhs=zh[:, zoff : zoff + RB * W],
                        start=False,
                        stop=True,
                    )
                    joff = (blk - g0) * RB * W
                    # PSUM->SBUF with the free affine scale undoing SCL,
                    # emitting fp16 (halves output DMA traffic).
                    nc.scalar.mul(
                        out=ot[:, joff : joff + RB * W], in_=ps[:], mul=1.0 / SCL
                    )
                    if blk == g0 + gsz - 1:
                        n = RB * W
                        nc.scalar.dma_start(
                            out=yflat[:, g0 * n : (g0 + gsz) * n], in_=ot[:]
                        )
    nc.finalize()
    return nc


_NC_CACHE = {}


def _get_nc():
    if "nc" not in _NC_CACHE:
        _NC_CACHE["nc"] = build_nc()
    return _NC_CACHE["nc"]


def _pad_images(x):
    """(B, I, H, W) fp32 -> fp16 (stride WP16, col C16) and fp8e4
    (stride WP8, col C8) zero-padded images."""
    xp16 = np.zeros((B, I, HP, WP16), np.float16)
    xp16[:, :, 1 : 1 + H, C16 : C16 + W] = x.astype(np.float16)
    xp8 = np.zeros((B, I, HP, WP8), ml_dtypes.float8_e4m3)
    xp8[:, :, 1 : 1 + H, C8 : C8 + W] = x.astype(ml_dtypes.float8_e4m3)
    return (
        np.ascontiguousarray(xp16.reshape(B, I, HP * WP16)),
        np.ascontiguousarray(xp8.reshape(B, I, HP * WP8)),
    )


def run(inputs, trace=False, **kw):
    """Run on hardware; returns (y, BassKernelResults)."""
    x = np.asarray(inputs["x"], np.float32)
    assert x.shape == (B, I, H, W), x.shape
    w16, w8, rh = synth_weights(
        inputs["core"], inputs["periphery"], inputs["threshold"], inputs["scale"]
    )
    xp16, xp8 = _pad_images(x)
    ratios = np.full((128, 1), rh, np.float32)
    nc = _get_nc()
    in_maps = [
        {
            "x16": xp16[c * BPC : (c + 1) * BPC],
            "x8": xp8[c * BPC : (c + 1) * BPC],
            "w16": w16,
            "w8": w8,
            "r": ratios,
        }
        for c in range(NCORES)
    ]
    res = run_bass_kernel_spmd(nc, in_maps, list(range(NCORES)), trace=trace, **kw)
    y = np.concatenate(
        [res.results[c]["y"].astype(np.float32) for c in range(NCORES)], axis=0
    )
    return y, res


def kernel(**inputs) -> np.ndarray:
    y, _ = run(inputs)
    return y


# revision 3
# speedup vs baseline: 1.3345x; 1.0760x over previous
"""CTConv2d Trainium2 kernel.

Computes y = conv2d(x, w) where w (O,I,3,3) is synthesized on host from
core/periphery/threshold/scale (tiny tensors), and the conv runs on 8
NeuronCores, data-parallel over batch (32 images -> 4 per core).

Device kernel (per core): the synthesized weight is
    w[o,i,dh,dw] = c[o,i] (center) / c*gate (cg) * p[dh,dw] (periphery),
so per 4-row output block the PE runs 5 matmuls (N=448 into fp32 PSUM):
  - 1 fp16 matmul for the center tap (accuracy-critical, ~93% of the
    output energy), moving operand = the raw fp16 image.
  - 4 fp8e4 DoubleRow matmuls, each covering a tap PAIR in one pass
    (DoubleRow packs 2 fp8 weights per PE cell, virtual K=256):
    top corners (dh=-1, dw=+-1), bottom corners (dh=+1, dw=+-1),
    vertical pair (dh=+-1, dw=0), horizontal pair (dh=0, dw=+-1).
    The moving operand is a hand-built 4D access pattern over the
    zero-padded fp8 image whose dim-1 is the pair: stride 2 elements
    for the +-dw pairs, 2 rows for the +-dh pair.
All weights are pre-scaled by S=1024 so the tiny fp8 weights (~4e-3)
land in e4m3's normal range; the grouped PSUM->SBUF copy on the scalar
engine compensates with its free affine scale (1/S) and emits fp16,
halving the output DMA. Per-image input: the raw fp16 image (center)
plus a zero-padded fp8e4 image (stride 120) for the DoubleRow taps,
chunk-loaded with the next image's chunks interleaved into the block
loop so prefetch never competes with the current image's tail chunks.
Accumulation is fp32; absmax rel err ~3e-3 vs the fp32 reference.
"""

import os
import sys

# The grading/bench environment may pin JAX_PLATFORMS=cpu for the jax
# reference; this kernel needs the axon/neuron PJRT backend.
if os.environ.get("JAX_PLATFORMS") == "cpu":
    del os.environ["JAX_PLATFORMS"]

for _p in ("/opt/trn_rl_repo",):
    if os.path.isdir(_p) and _p not in sys.path:
        sys.path.append(_p)

import numpy as np
import ml_dtypes
import bass_rust

import concourse.bass as bass
import concourse.mybir as mybir
from concourse import bacc
from concourse.bass_utils import run_bass_kernel_spmd
from concourse.tile import TileContext

O = 128
I = 128
B = 32
H = 112
W = 112
NCORES = 8
BPC = B // NCORES  # images per core
HP = H + 2  # fp8 padded rows (interior at row 1)
WP8 = 120  # fp8 image row stride (2*WP8 % 16 == 0 for the DR pair step)
C8 = 4  # fp8 interior column offset
RB = 4  # output rows per block (N = RB*W = 448 <= 512)
NBLK = H // RB  # 28
GRP = 4  # blocks per PSUM tile / grouped copy / output DMA
NG = NBLK // GRP  # 7 groups per image
SCL = 1024.0  # weight pre-scale so fp8 weights sit in e4m3 normal range
BANK = 512  # PSUM bank stride in fp32 elements
# image-load chunks (row ranges); first chunk small so block 0 starts asap
CHUNKS8 = [(0, 6), (6, 18), (18, 50), (50, 82), (82, HP)]  # fp8 padded rows
CHUNKS16 = [(0, 5), (5, 17), (17, 49), (49, 81), (81, H)]  # raw x rows
# image-b block index at which chunk ci of image b+1 is issued
PREFETCH_AT = [3, 8, 13, 18, 23]
F32 = mybir.dt.float32
F16 = mybir.dt.float16
F8 = mybir.dt.float8e4

# DoubleRow tap pairs: (name, pair-dim kind, padded-row offset rel. to h0,
# col offset rel. to C8): pair stride 2 elems for dw=+-1, 2 rows for dh=+-1
# slot0/slot1 weights are p[dh,dw] for the two paired taps.
DR_TAPS = [
    ((-1, -1), (-1, +1), "col", 0),  # top corners: row h0+1-1 = h0
    ((+1, -1), (+1, +1), "col", 2),  # bottom corners: row h0+2
    ((-1, 0), (+1, 0), "row", 0),  # vertical pair: rows h0, h0+2
    ((0, -1), (0, +1), "col", 1),  # horizontal pair: row h0+1
]

EPS = 1e-7


def synth_weights(core, periphery, threshold, scale):
    """Host-side weight synthesis.

    Returns (w16, w8):
      w16 (I, O) fp16 lhsT: center tap c*S.
      w8 (I, 4*2*O) fp8e4 lhsT, per DR_TAPS block t:
        [t*2*O + slot*O + o] = cg * p[tap_slot] * S.
    """
    c = np.asarray(core, np.float64)[:, :, 0, 0]  # (O, I)
    thr = np.asarray(threshold, np.float64)
    s = float(np.asarray(scale, np.float64)[0])
    p = np.asarray(periphery, np.float64)
    gate = 1.0 / (1.0 + np.exp(-s * (np.abs(c) - thr[:, None])))
    p_full = np.concatenate([p[:4], [1.0], p[4:]])  # (9,) taps row-major
    cg = c * gate

    def ptap(dh, dw):
        return p_full[(dh + 1) * 3 + (dw + 1)]

    w16 = np.ascontiguousarray((c * SCL).T.astype(np.float16))

    w8 = np.empty((4, 2, O, I), np.float64)
    for t, (tapA, tapB, _, _) in enumerate(DR_TAPS):
        w8[t, 0] = cg * ptap(*tapA) * SCL
        w8[t, 1] = cg * ptap(*tapB) * SCL
    w8 = w8.transpose(3, 0, 1, 2).reshape(I, 4 * 2 * O)
    w8 = np.clip(w8, -240.0, 240.0)
    return w16, np.ascontiguousarray(w8.astype(ml_dtypes.float8_e4m3))


def _dr_rhs(img8v, h0, kind, roff):
    """Moving-operand AP for one DoubleRow pair MM at block row h0.

    kind='col': pair = cols (C8-1, C8+1) at padded row h0+roff.
    kind='row': pair = padded rows (h0, h0+2) at col C8.
    Free dims [pair:2, h:RB, w:W] -> free size 2*RB*W = 896.
    """
    if kind == "col":
        sl = img8v[:, h0 + roff : h0 + roff + RB, C8 - 1 : C8 + 1 + W]
        pair = [2, 2]
    else:
        sl = img8v[:, h0 : h0 + RB + 2, C8 : C8 + W]
        pair = [2 * WP8, 2]
    rhs = sl.copy()
    rhs.ap = bass_rust.VecI64Pair([list(sl.ap[0]), pair, [WP8, RB], [1, W]])
    return rhs


def build_nc():
    nc = bacc.Bacc(None)
    x16_d = nc.dram_tensor("x16", [BPC, I, H * W], F16, kind="ExternalInput")
    x8_d = nc.dram_tensor("x8", [BPC, I, HP * WP8], F8, kind="ExternalInput")
    w16_d = nc.dram_tensor("w16", [I, O], F16, kind="ExternalInput")
    w8_d = nc.dram_tensor("w8", [I, 4 * 2 * O], F8, kind="ExternalInput")
    y_d = nc.dram_tensor("y", [BPC, O, H, W], F16, kind="ExternalOutput")

    with TileContext(nc) as tc, tc.tile_pool(name="persist", bufs=1) as persist:
        w16t = persist.tile([I, O], F16, name="w16t", tag="w16t")
        w8t = persist.tile([I, 4 * 2 * O], F8, name="w8t", tag="w8t")
        # weights on the ACT ring (idle at start; the out DMAs come later),
        # image chunks on the SP ring -> the two loads run in parallel.
        nc.scalar.dma_start(out=w16t[:], in_=w16_d[:])
        nc.scalar.dma_start(out=w8t[:], in_=w8_d[:])

        imgs16 = []
        imgs8 = []
        for ib in range(2):
            imgs16.append(
                persist.tile([128, H * W], F16, name=f"i16_{ib}", tag=f"i16_{ib}")
            )
            imgs8.append(
                persist.tile([128, HP * WP8], F8, name=f"i8_{ib}", tag=f"i8_{ib}")
            )

        # HAM warmup: the PE clock gate sits at 1.2 GHz until ~3.4us of
        # sustained matmul activity. A dependency-free burst right after
        # engine boot flips it to 2.4 GHz before the first real matmul
        # (which waits on the weight/first-chunk DMA receipt chain anyway).
        warm = persist.tile([128, 640], F16, name="warm", tag="warm")
        nc.vector.memset(warm[:], 0.0)

        def load_chunk(b, ci):
            i16 = imgs16[b % 2]
            i8 = imgs8[b % 2]
            r0, r1 = CHUNKS16[ci]
            nc.sync.dma_start(
                out=i16[:, r0 * W : r1 * W], in_=x16_d[b][:, r0 * W : r1 * W]
            )
            r0, r1 = CHUNKS8[ci]
            nc.sync.dma_start(
                out=i8[:, r0 * WP8 : r1 * WP8], in_=x8_d[b][:, r0 * WP8 : r1 * WP8]
            )

        with (
            tc.tile_pool(name="psum", bufs=2, space="PSUM") as psum_pool,
            tc.tile_pool(name="outp", bufs=3) as out_pool,
        ):
            for ci in range(len(CHUNKS8)):
                load_chunk(0, ci)
            for k in range(10):
                pw = psum_pool.tile([128, GRP * BANK], F32, name="pw", tag="ps")
                nc.tensor.matmul(
                    out=pw[:, 0:512],
                    lhsT=warm[:, 0:128],
                    rhs=warm[:, 128:640],
                    start=True,
                    stop=True,
                )
            for b in range(BPC):
                img16v = imgs16[b % 2].rearrange("p (h w) -> p h w", w=W)
                img8v = imgs8[b % 2].rearrange("p (h w) -> p h w", w=WP8)
                yflat = y_d[b].rearrange("o h w -> o (h w)")
                ps = None
                ot = None
                for blk in range(NBLK):
                    if b + 1 < BPC and blk in PREFETCH_AT:
                        load_chunk(b + 1, PREFETCH_AT.index(blk))
                    g = blk // GRP  # output group
                    j = blk % GRP  # slot in group
                    if j == 0:
                        ps = psum_pool.tile([128, GRP * BANK], F32, name="ps")
                        ot = out_pool.tile([128, GRP * RB * W], F16, name="ot")
                    h0 = blk * RB
                    pslice = ps[:, j * BANK : j * BANK + RB * W]
                    nc.tensor.matmul(
                        out=pslice,
                        lhsT=w16t[:],
                        rhs=img16v[:, h0 : h0 + RB, 0:W],
                        start=True,
                        stop=False,
                    )
                    for t, (_, _, kind, roff) in enumerate(DR_TAPS):
                        nc.tensor.matmul(
                            out=pslice,
                            lhsT=w8t[:, t * 2 * O : (t + 1) * 2 * O].rearrange(
                                "p (pair o) -> p pair o", pair=2
                            ),
                            rhs=_dr_rhs(img8v, h0, kind, roff),
                            start=False,
                            stop=(t == len(DR_TAPS) - 1),
                            perf_mode=mybir.MatmulPerfMode.DoubleRow,
                        )
                    last_group = b == BPC - 1 and g == NG - 1
                    if j == GRP - 1:
                        n = RB * W
                        ps4 = ps.rearrange("p (g c) -> p g c", c=BANK)
                        ot3 = ot.rearrange("p (g c) -> p g c", c=n)
                        if not last_group:
                            # grouped PSUM->SBUF copy: one ACTIVATE over the 4
                            # banks, free affine scale undoing SCL, fp16 out.
                            nc.scalar.mul(
                                out=ot3[:], in_=ps4[:, :, 0:n], mul=1.0 / SCL
                            )
                            nc.scalar.dma_start(
                                out=yflat[:, g * GRP * n : (g + 1) * GRP * n],
                                in_=ot[:],
                            )
                        else:
                            # split the very last copy/DMA so the tail after
                            # the final matmul is one 448-col copy + small DMA
                            nc.scalar.mul(
                                out=ot3[:, 0:3], in_=ps4[:, 0:3, 0:n], mul=1.0 / SCL
                            )
                            nc.scalar.dma_start(
                                out=yflat[:, g * GRP * n : (g * GRP + 3) * n],
                                in_=ot[:, 0 : 3 * n],
                            )
                    elif last_group and j == GRP - 2:
                        pass  # (block 26 handled with the j==GRP-1 branch)
                if b == BPC - 1:
                    # final block's copy + DMA (emitted after the loop so the
                    # ps/ot handles are still in scope)
                    n = RB * W
                    ps4 = ps.rearrange("p (g c) -> p g c", c=BANK)
                    ot3 = ot.rearrange("p (g c) -> p g c", c=n)
                    nc.scalar.mul(
                        out=ot3[:, 3:4], in_=ps4[:, 3:4, 0:n], mul=1.0 / SCL
                    )
                    nc.scalar.dma_start(
                        out=yflat[:, (NBLK - 1) * n : NBLK * n],
                        in_=ot[:, 3 * n : 4 * n],
                    )
    nc.finalize()
    return nc


_NC_CACHE = {}


def _get_nc():
    if "nc" not in _NC_CACHE:
        _NC_CACHE["nc"] = build_nc()
    return _NC_CACHE["nc"]


def _prep_images(x):
    """(B, I, H, W) fp32 -> raw fp16 image and zero-padded fp8e4 image."""
    x16 = np.ascontiguousarray(x.astype(np.float16).reshape(B, I, H * W))
    xp8 = np.zeros((B, I, HP, WP8), ml_dtypes.float8_e4m3)
    xp8[:, :, 1 : 1 + H, C8 : C8 + W] = x.astype(ml_dtypes.float8_e4m3)
    return x16, np.ascontiguousarray(xp8.reshape(B, I, HP * WP8))


def run(inputs, trace=False, **kw):
    """Run on hardware; returns (y, BassKernelResults)."""
    x = np.asarray(inputs["x"], np.float32)
    assert x.shape == (B, I, H, W), x.shape
    w16, w8 = synth_weights(
        inputs["core"], inputs["periphery"], inputs["threshold"], inputs["scale"]
    )
    x16, x8 = _prep_images(x)
    nc = _get_nc()
    in_maps = [
        {
            "x16": x16[c * BPC : (c + 1) * BPC],
            "x8": x8[c * BPC : (c + 1) * BPC],
            "w16": w16,
            "w8": w8,
        }
        for c in range(NCORES)
    ]
    res = run_bass_kernel_spmd(nc, in_maps, list(range(NCORES)), trace=trace, **kw)
    y = np.concatenate(
        [res.results[c]["y"].astype(np.float32) for c in range(NCORES)], axis=0
    )
    return y, res


def kernel(**inputs) -> np.ndarray:
    y, _ = run(inputs)
    return y
